# revision 1
# baseline (speedup 1.0000x reference)
"""BinaryLinear kernel for Trainium2 (8 NeuronCores, SPMD).

Computes  out = sign(x) @ sign(W)^T * alpha  for
x: [8192, 2048] f32, W: [2048, 2048] f32, alpha: [1] f32.

Strategy: data-parallel over the token dim (8 shards of 1024 tokens);
W replicated. Host side pre-transposes x-shards and W so the device
sees the contraction dim (in_features) on partitions. On device:
sign() both operands into resident bf16 SBUF buffers (+-1 is exact in
bf16, and accumulation of <=2048 +-1 terms is exact in fp32 PSUM),
then a dense PE matmul, scale by alpha, write out.
"""

import numpy as np

import concourse.bass as bass
import concourse.tile as tile
from concourse import bacc, mybir
from concourse.bass_utils import run_bass_kernel_spmd

N_CORES = 8
NTOK = 8192
INF = 2048
OUTF = 2048
TPC = NTOK // N_CORES  # tokens per core (1024)
P = 128
KT = INF // P  # 16 contraction tiles
MT = TPC // P  # 8 token tiles per core
NTS = 512  # out_features per matmul (one PSUM bank)
NT = OUTF // NTS  # 4

F32 = mybir.dt.float32
BF16 = mybir.dt.bfloat16

_compiled = None
LAST_RESULT = None  # BassKernelResults of the most recent run (for profiling)


def _build():
    nc = bacc.Bacc(
        "TRN2",
        target_bir_lowering=False,
        debug=False,
        num_devices=N_CORES,
    )
    xt = nc.dram_tensor("xt", [INF, TPC], F32, kind="ExternalInput").ap()
    wt = nc.dram_tensor("wt", [INF, OUTF], F32, kind="ExternalInput").ap()
    al = nc.dram_tensor("alpha", [P, 1], F32, kind="ExternalInput").ap()
    out = nc.dram_tensor("out", [TPC, OUTF], F32, kind="ExternalOutput").ap()

    with tile.TileContext(nc) as tc:
        with (
            tc.tile_pool(name="res", bufs=1) as res,
            tc.tile_pool(name="wload", bufs=3) as wload,
            tc.tile_pool(name="xload", bufs=3) as xload,
            tc.tile_pool(name="psum", bufs=8, space="PSUM") as ppool,
            tc.tile_pool(name="outp", bufs=4) as outp,
        ):
            alpha_t = res.tile([P, 1], F32)
            nc.sync.dma_start(alpha_t[:], al)

            # Resident sign() buffers: bw [128, 16*2048] bf16, bx [128, 16*1024] bf16
            bw = res.tile([P, KT * OUTF], BF16)
            bx = res.tile([P, KT * TPC], BF16)

            for k in range(KT):
                wf = wload.tile([P, OUTF], F32)
                nc.sync.dma_start(wf[:], wt[k * P : (k + 1) * P, :])
                # ACT engine: sign(w) -> bf16
                nc.scalar.sign(bw[:, k * OUTF : (k + 1) * OUTF], wf[:])

                xf = xload.tile([P, TPC], F32)
                nc.sync.dma_start(xf[:], xt[k * P : (k + 1) * P, :])
                # DVE: sign(x) as (x > 0) * 2 - 1 (no exact zeros in inputs)
                xg = xload.tile([P, TPC], F32)
                nc.vector.tensor_scalar(
                    xg[:], xf[:], 0.0, None, op0=mybir.AluOpType.is_gt
                )
                nc.vector.tensor_scalar(
                    bx[:, k * TPC : (k + 1) * TPC],
                    xg[:],
                    2.0,
                    -1.0,
                    op0=mybir.AluOpType.mult,
                    op1=mybir.AluOpType.add,
                )

            for m in range(MT):
                for n in range(NT):
                    ps = ppool.tile([P, NTS], F32)
                    for k in range(KT):
                        nc.tensor.matmul(
                            ps[:],
                            bx[:, k * TPC + m * P : k * TPC + (m + 1) * P],
                            bw[:, k * OUTF + n * NTS : k * OUTF + (n + 1) * NTS],
                            start=(k == 0),
                            stop=(k == KT - 1),
                        )
                    ot = outp.tile([P, NTS], F32)
                    nc.scalar.activation(
                        ot[:],
                        ps[:],
                        mybir.ActivationFunctionType.Copy,
                        scale=alpha_t[:],
                    )
                    nc.sync.dma_start(
                        out[m * P : (m + 1) * P, n * NTS : (n + 1) * NTS], ot[:]
                    )

    nc.compile()
    return nc


def kernel(x, weight, alpha):
    global _compiled, LAST_RESULT
    if _compiled is None:
        _compiled = _build()
    nc = _compiled

    x = np.asarray(x, dtype=np.float32)
    weight = np.asarray(weight, dtype=np.float32)
    alpha = np.asarray(alpha, dtype=np.float32)

    wt = np.ascontiguousarray(weight.T)
    alv = np.full((P, 1), alpha.reshape(-1)[0], dtype=np.float32)
    in_maps = []
    for c in range(N_CORES):
        xs = np.ascontiguousarray(x[c * TPC : (c + 1) * TPC, :].T)
        in_maps.append({"xt": xs, "wt": wt, "alpha": alv})

    LAST_RESULT = run_bass_kernel_spmd(nc, in_maps, list(range(N_CORES)))
    outs = [LAST_RESULT.results[c]["out"] for c in range(N_CORES)]
    return np.concatenate(outs, axis=0)


# revision 4
# speedup vs baseline: 1.0983x; 1.0983x over previous
"""BinaryLinear kernel for Trainium2 (8 NeuronCores, SPMD).

Computes  out = sign(x) @ sign(W)^T * alpha  for
x: [8192, 2048] f32, W: [2048, 2048] f32, alpha: [1] f32.

Strategy: data-parallel over the token dim (8 shards of 1024 tokens);
W replicated. Host side pre-transposes x-shards and W so the device
sees the contraction dim (in_features) on partitions. On device:
sign() both operands into resident bf16 SBUF buffers (+-1 is exact in
bf16, and accumulation of <=2048 +-1 terms is exact in fp32 PSUM),
then a dense PE matmul, scale by alpha, write out.

Scheduling: n-outer / k-middle / m-inner with 8 live PSUM banks so the
PE consumes each contraction tile against the full token dim as soon
as it is loaded+signed. DMA issue order matches consumption order
(x[k] and W[k,n0] interleaved, then W[k,n1] chunks, then n2/n3 as
single large strided DMAs).
"""

import numpy as np

import concourse.bass as bass
import concourse.tile as tile
from concourse import bacc, mybir
from concourse.bass_utils import run_bass_kernel_spmd

N_CORES = 8
NTOK = 8192
INF = 2048
OUTF = 2048
TPC = NTOK // N_CORES  # tokens per core (1024)
P = 128
KT = INF // P  # 16 contraction tiles
MT = TPC // P  # 8 token tiles per core
NTS = 512  # out_features per matmul (one PSUM bank)
NT = OUTF // NTS  # 4

F32 = mybir.dt.float32
BF16 = mybir.dt.bfloat16

_compiled = None
LAST_RESULT = None  # BassKernelResults of the most recent run (for profiling)


def _build():
    nc = bacc.Bacc(
        "TRN2",
        target_bir_lowering=False,
        debug=False,
        num_devices=N_CORES,
    )
    xt = nc.dram_tensor("xt", [INF, TPC], F32, kind="ExternalInput").ap()
    wt = nc.dram_tensor("wt", [INF, OUTF], F32, kind="ExternalInput").ap()
    al = nc.dram_tensor("alpha", [P, 1], F32, kind="ExternalInput").ap()
    out = nc.dram_tensor("out", [TPC, OUTF], F32, kind="ExternalOutput").ap()

    # [128, k, .] / [128, m, .] views of the DRAM tensors
    wt_r = wt.rearrange("(k p) c -> p k c", p=P)  # [128, 16, 2048]
    xt_r = xt.rearrange("(k p) c -> p k c", p=P)  # [128, 16, 1024]
    out_r = out.rearrange("(m p) c -> p m c", p=P)  # [128, 8, 2048]

    with tile.TileContext(nc) as tc:
        with (
            tc.tile_pool(name="res", bufs=1) as res,
            tc.tile_pool(name="wload", bufs=4) as wload,
            tc.tile_pool(name="wbig", bufs=1) as wbig,
            tc.tile_pool(name="xload", bufs=3) as xload,
            tc.tile_pool(name="psum", bufs=8, space="PSUM") as ppool,
            tc.tile_pool(name="outp", bufs=1) as outp,
        ):
            alpha_t = res.tile([P, 1], F32)
            nc.sync.dma_start(alpha_t[:], al)

            # Resident sign() buffers (bf16)
            bw = res.tile([P, KT, OUTF], BF16)  # 64 KB/partition
            bx = res.tile([P, KT, TPC], BF16)  # 32 KB/partition

            def sign_w(k, n, wf):
                # ACT: sign(w chunk) -> bf16
                nc.scalar.sign(bw[:, k, n * NTS : (n + 1) * NTS], wf[:])

            def load_sign_w_chunk(k, n):
                wf = wload.tile([P, NTS], F32)
                nc.sync.dma_start(wf[:], wt_r[:, k, n * NTS : (n + 1) * NTS])
                sign_w(k, n, wf)

            # ---- load + sign phase (issue order == consumption order) ----
            # x[k] + W[k, n0] interleaved, then W[k, n1] chunks.
            for k in range(KT):
                xf = xload.tile([P, TPC], F32)
                nc.sync.dma_start(xf[:], xt_r[:, k, :])
                # DVE: sign(x) as (x > 0) -> {1,0} bf16, then in-place *2-1
                nc.vector.tensor_scalar(
                    bx[:, k, :], xf[:], 0.0, None, op0=mybir.AluOpType.is_gt
                )
                nc.vector.tensor_scalar(
                    bx[:, k, :],
                    bx[:, k, :],
                    2.0,
                    -1.0,
                    op0=mybir.AluOpType.mult,
                    op1=mybir.AluOpType.add,
                )
                load_sign_w_chunk(k, 0)
            for k in range(KT):
                load_sign_w_chunk(k, 1)
            for n in (2, 3):
                wf = wbig.tile([P, KT, NTS], F32)
                nc.sync.dma_start(wf[:], wt_r[:, :, n * NTS : (n + 1) * NTS])
                for k in range(KT):
                    sign_w(k, n, wf[:, k, :])

            # ---- matmul phase: n-outer, k-middle, m-inner (8 psum banks) ----
            for n in range(NT):
                obuf = outp.tile([P, MT, NTS], F32)
                pss = []
                for m in range(MT):
                    pss.append(ppool.tile([P, NTS], F32, name="ps", tag="ps"))
                for k in range(KT):
                    for m in range(MT):
                        nc.tensor.matmul(
                            pss[m][:],
                            bx[:, k, m * P : (m + 1) * P],
                            bw[:, k, n * NTS : (n + 1) * NTS],
                            start=(k == 0),
                            stop=(k == KT - 1),
                        )
                for m in range(MT):
                    # DVE: scale by alpha while draining PSUM -> SBUF
                    nc.vector.tensor_scalar_mul(obuf[:, m, :], pss[m][:], alpha_t[:])
                nc.sync.dma_start(out_r[:, :, n * NTS : (n + 1) * NTS], obuf[:])

    nc.compile()
    return nc


def kernel(x, weight, alpha):
    global _compiled, LAST_RESULT
    if _compiled is None:
        _compiled = _build()
    nc = _compiled

    x = np.asarray(x, dtype=np.float32)
    weight = np.asarray(weight, dtype=np.float32)
    alpha = np.asarray(alpha, dtype=np.float32)

    wt = np.ascontiguousarray(weight.T)
    alv = np.full((P, 1), alpha.reshape(-1)[0], dtype=np.float32)
    in_maps = []
    for c in range(N_CORES):
        xs = np.ascontiguousarray(x[c * TPC : (c + 1) * TPC, :].T)
        in_maps.append({"xt": xs, "wt": wt, "alpha": alv})

    LAST_RESULT = run_bass_kernel_spmd(nc, in_maps, list(range(N_CORES)))
    outs = [LAST_RESULT.results[c]["out"] for c in range(N_CORES)]
    return np.concatenate(outs, axis=0)
